# revision 8
# baseline (speedup 1.0000x reference)
"""Trainium2 Bass kernel for nn_Actor (3 grouped conv1d blocks + dense + tanh).

Sharding: column-parallel across 8 cores. Core j owns input channels
{2j, 2j+1}; every conv is grouped (depthwise x8 filters), so that
slice owns contiguous channel blocks through the whole net:
  conv1 out-ch [16j,16j+16), conv2 out-ch [128j,128j+128),
  conv3 out-ch [1024j, 1024j+1024), and rows {l*8192 + ch} of W.
Each core computes partial dense outputs; the host sums them, adds bd
and applies tanh.

Pipeline design (all bf16 compute, f32 psum):
- conv1 out tmp1 [128p = (c1*8 + bg), (l1, b8)].
- i2 [80p = (c1*5 + k), (bg, l2, b8)] via 8 DMAs (one per bg).
- x2r [128p = P2(c2), (bg, l2, b8)] 960-stride cols.
- conv3 im2col i3 [80p = (c*5+k), 7680]: i3[(c,k), col] = x2r[P(c), col+8k]
  (UNtrimmed copy: tap shift is a flat column offset, so each (partition,
  tap) is ONE ~15KB contiguous DMA segment; 10 DMA instructions per group).
  Valid window per bg block: cols bg*960 + [0, 928).
- conv3: per (group, bg): 2 MMs (512+416) into a [128, 1024] 2-bank psum
  tile (3-buf rotation), ONE relu [128, 928] per tile (alternating
  Act/DVE) -> zf [128, 7680] (bufs=4).
- dense: quad-MMs reading zf(g-2) (2-group-old => RAW waits ancient),
  interleaved ~4 per bg-tile so PE wait-processing is always covered by
  wait-free MMs; 4 col-tile accumulators in ONE psum bank.
- software-pipelined loop body: next iteration's conv1/i2/conv2 and
  i3(g0,g1) prefetch are emitted between dense tail groups 6 and 7, so
  the PE stream crosses iteration boundaries without a gap.
"""

import ml_dtypes
import numpy as np

import concourse.bass as bass
import concourse.tile as tile
from concourse import bacc
from concourse import mybir
from concourse.bass_utils import run_bass_kernel_spmd

B = 64
L = 128
C = 16
FILTERS = 8
K = 5
N_CORES = 8

L1 = L - K + 1      # 124
L2 = L1 - K + 1     # 120
L3 = L2 - K + 1     # 116

C0 = C // N_CORES           # 2    input cols per core
C1 = C0 * FILTERS           # 16   conv1 out-ch per core
C2 = C1 * FILTERS           # 128  conv2 out-ch per core
C3 = C2 * FILTERS           # 1024 conv3 out-ch per core
G3 = C2 // C1               # 8    conv3 groups of 16 in-ch
BG = 8                      # batch groups
B8 = B // BG                # 8

NB1 = L1 * B8               # 992   conv1 free (l1, b8)
NB2 = L2 * B                # 7680  conv2 cols (bg, l2, b8)
SB3 = L2 * B8               # 960   per-bg col stride
VB3 = L3 * B8               # 928   valid cols per bg block

F32 = mybir.dt.float32
BF16 = mybir.dt.bfloat16

_CACHE = {}


def _build_nc(reps=1, mode='full'):
    """Build the SPMD Bass program (same on all 8 cores).

    reps>1 wraps the pipeline in a device-side loop (used only for
    timing by wall-clock differencing).
    """
    nc = bacc.Bacc("TRN2", target_bir_lowering=False, debug=False)

    # cs (f32): col0 b1p [(c1*8+bg)], col1 b2p [c2], cols 2..9 b3p per group
    # sb (bf16): s1 [80,128] @0, s2 [80,128] @128, s3 8x[80,128] @256
    a1 = nc.declare_dram_parameter("a1", [80, NB1], BF16, isOutput=False)
    cs = nc.declare_dram_parameter("cs", [128, 10], F32, isOutput=False)
    sb = nc.declare_dram_parameter("sb", [80, 256 + G3 * 128], BF16, isOutput=False)
    wt = nc.declare_dram_parameter("wt", [128, G3 * L3 * 2], BF16, isOutput=False)
    out = nc.declare_dram_parameter("out", [128, 4 * B], F32, isOutput=True)

    do_mm = mode not in ('empty', 'conv2stop', 'i3only')
    do_relu3 = mode not in ('norelu',)
    do_dense = mode not in ('nodense',)

    with tile.TileContext(nc) as tc:
        with (
            tc.tile_pool(name="consts", bufs=1) as consts,
            tc.tile_pool(name="work", bufs=1) as work,
            tc.tile_pool(name="i3pool", bufs=4) as i3pool,
            tc.tile_pool(name="zpool", bufs=4) as zpool,
            tc.tile_pool(name="psum", bufs=3, space=bass.MemorySpace.PSUM) as psum,
            tc.tile_pool(name="psumd", bufs=1, space=bass.MemorySpace.PSUM) as psumd,
        ):
            # ---- constants ----
            a1_t = consts.tile([80, NB1], BF16)
            cs_t = consts.tile([128, 10], F32)
            sb_t = consts.tile([80, 256 + G3 * 128], BF16)
            nc.sync.dma_start(a1_t[:], a1[:])
            nc.scalar.dma_start(cs_t[:], cs[:])
            nc.sync.dma_start(sb_t[:], sb[:])
            WQ = G3 * L3 * 2 // 4  # 464
            wt_ts = []
            for q in range(4):
                wq = consts.tile([128, WQ], BF16, tag=f"wt{q}")
                (nc.scalar if q % 2 else nc.sync).dma_start(
                    wq[:], wt[:, q * WQ:(q + 1) * WQ])
                wt_ts.append(wq)

            # persistent work tiles (single-buffered; WAR handled by sems)
            tmp1 = work.tile([128, NB1], BF16)
            i2 = work.tile([80, NB2], BF16)
            x2r = work.tile([C2, NB2], BF16)

            # DMA ring alternation
            rings = [nc.sync, nc.scalar]
            ring_i = [0]

            def dma(dst, src):
                rings[ring_i[0]].dma_start(dst, src)
                ring_i[0] ^= 1

            # relu engine alternation: Act / DVE
            relu_i = [0]

            def relu(dst, src, bias_ap):
                if relu_i[0] == 0:
                    nc.scalar.activation(dst, src,
                                         mybir.ActivationFunctionType.Relu,
                                         bias=bias_ap)
                else:
                    nc.vector.tensor_scalar(dst, src, bias_ap, 0.0,
                                            mybir.AluOpType.add,
                                            mybir.AluOpType.max)
                relu_i[0] ^= 1

            def prologue():
                """conv1 + i2 im2col + conv2 + issue i3(g0, g1)."""
                # conv1: 2 MMs into one 2-bank psum tile, one relu
                if mode != 'empty':
                    p1 = psum.tile([128, 1024], F32, tag="pchunk", name="p1")
                    nc.tensor.matmul(p1[:, 0:512], sb_t[0:80, 0:128],
                                     a1_t[:, 0:512], start=True, stop=True)
                    nc.tensor.matmul(p1[:, 512:NB1], sb_t[0:80, 0:128],
                                     a1_t[:, 512:NB1], start=True, stop=True)
                    relu(tmp1[:], p1[:, 0:NB1], cs_t[:, 0:1])

                    # i2 im2col: ONE DMA per bg
                    for bg in range(BG):
                        t1b = tmp1[bg:128, 0:NB1]
                        src = bass.AP(t1b.tensor, t1b.offset,
                                      [[t1b.ap[0][0] * 8, C1], [B8, K],
                                       [1, SB3]])
                        dma(i2[0:80, bg * SB3:(bg + 1) * SB3], src)

                    # conv2: 15 MMs, relu per 1024 (512 for the tail)
                    for t in range(8):
                        n = 1024 if t < 7 else 512
                        p2 = psum.tile([128, 1024], F32, tag="pchunk",
                                       name="p2")
                        nc.tensor.matmul(p2[:, 0:512], sb_t[0:80, 128:256],
                                         i2[:, t * 1024:t * 1024 + 512],
                                         start=True, stop=True)
                        if n == 1024:
                            nc.tensor.matmul(
                                p2[:, 512:1024], sb_t[0:80, 128:256],
                                i2[:, t * 1024 + 512:(t + 1) * 1024],
                                start=True, stop=True)
                        relu(x2r[:, t * 1024:t * 1024 + n], p2[:, 0:n],
                             cs_t[:, 1:2])
                if mode not in ('empty', 'conv2stop'):
                    issue_i3(0)
                    issue_i3(1)

            i3s = {}

            def issue_i3(g):
                # 10 DMAs (tap k x half e): dst i3 rows (c*5+k), c=8e+d;
                # i3[(c,k), col] = x2r[P(c), col + 8k] -- ONE contiguous
                # ~15KB segment per dst partition.
                i3 = i3pool.tile([80, NB2], BF16, tag="i3", name="i3")
                xb = x2r[0:128, 0:NB2]
                xp = xb.ap[0][0]
                ib = i3[0:80, 0:NB2]
                pp = ib.ap[0][0]
                pbase = (g % 4) + 32 * (g // 4)
                for k in range(K):
                    for e in range(2):
                        n = NB2 - 8 * k
                        src = bass.AP(xb.tensor,
                                      xb.offset + (pbase + 64 * e) * xp + 8 * k,
                                      [[4 * xp, 8], [1, n]])
                        dst = bass.AP(ib.tensor,
                                      ib.offset + (40 * e + k) * pp,
                                      [[5 * pp, 8], [1, n]])
                        dma(dst, src)
                i3s[g] = i3

            # dense: quad-MMs (4 l x 2 actions = 8 psum rows, moving
            # cols (bg, 4, b8) = 256); 4 col-tile accumulators in ONE
            # psum bank (rows 32*tj..32*tj+8, cols 0..256).
            NQ = L3 // 4                 # 29 quads per group
            per_tile = G3 * NQ // 4      # 58 MMs per col-tile
            pd = psumd.tile([128, 512], F32, tag="pd", name="pd")
            tile_seen = [0, 0, 0, 0]
            qcount = [0]

            def emit_dense_quad(zf, g):
                lq = qcount[0] % NQ
                qcount[0] += 1
                zb = zf[0:128, 0:1]
                zp, zf0 = zb.ap[0], zb.offset
                tj = (g * NQ + lq) % 4
                wcol = 232 * (g % 2) + 8 * lq
                mv = bass.AP(zb.tensor, zf0 + 4 * lq * B8,
                             [zp, [SB3, BG], [B8, 4], [1, B8]])
                nc.tensor.matmul(pd[32 * tj:32 * tj + 8, 0:4 * B],
                                 wt_ts[g // 2][:, wcol:wcol + 8],
                                 mv,
                                 start=(tile_seen[tj] == 0),
                                 stop=(tile_seen[tj] == per_tile - 1),
                                 tile_position=(0, 32 * tj))
                tile_seen[tj] += 1

            def conv3_group(g, pend):
                """conv3 MMs + relu for group g; interleave dense quads of
                pend = (zf, g-2)."""
                i3 = i3s.pop(g)
                if do_relu3:
                    zf = zpool.tile([C2, NB2], BF16, tag="zf", name="zf")
                else:
                    zf = x2r  # junk moving data for the norelu timing probe
                # quads per bg slot: 4,4,4,4,4,3,3,3 = 29
                for bg in range(BG):
                    p3 = psum.tile([128, 1024], F32, tag="pchunk", name="p3")
                    c0 = bg * SB3
                    nc.tensor.matmul(p3[:, 0:512],
                                     sb_t[:, 256 + g * 128:256 + (g + 1) * 128],
                                     i3[:, c0:c0 + 512], start=True, stop=True)
                    nc.tensor.matmul(p3[:, 512:VB3],
                                     sb_t[:, 256 + g * 128:256 + (g + 1) * 128],
                                     i3[:, c0 + 512:c0 + VB3],
                                     start=True, stop=True)
                    if do_relu3:
                        relu(zf[:, c0:c0 + VB3], p3[:, 0:VB3],
                             cs_t[:, 2 + g:3 + g])
                    if pend is not None and do_dense:
                        nq = 4 if bg < 5 else 3
                        for _ in range(nq):
                            emit_dense_quad(*pend)
                return zf

            rep_cm = (tc.For_i(0, reps, 1,
                               hint_engines=(mybir.EngineType.PE,
                                             mybir.EngineType.DVE,
                                             mybir.EngineType.Activation,
                                             mybir.EngineType.SP,
                                             mybir.EngineType.Pool))
                      if reps > 1 else None)

            def dense_tail(zf, g):
                for _ in range(NQ):
                    emit_dense_quad(zf, g)

            prologue()
            if rep_cm is not None:
                rep_cm.__enter__()

            if do_mm:
                zfs = {}
                for g in range(G3):
                    if g + 2 < G3 and mode != 'i3only':
                        issue_i3(g + 2)
                    pend = (zfs.pop(g - 2), g - 2) if g >= 2 else None
                    zfs[g] = conv3_group(g, pend)
                if do_dense:
                    dense_tail(zfs[6], 6)
                if rep_cm is not None:
                    prologue()   # next iteration's, overlaps dense tail
                if do_dense:
                    dense_tail(zfs[7], 7)
                zfs.clear()
            else:
                if mode == 'i3only':
                    for g in range(G3 - 2):
                        issue_i3(g + 2)
                    i3s.clear()
                if rep_cm is not None:
                    prologue()

            # ---- write partials ----
            out_t = work.tile([128, 4 * B], F32)
            if mode == 'full':
                for tj in range(4):
                    nc.vector.tensor_copy(out_t[32 * tj:32 * tj + 8, :],
                                          pd[32 * tj:32 * tj + 8, 0:4 * B])
            else:
                nc.gpsimd.memset(out_t[:], 0.0)
            nc.sync.dma_start(out[:], out_t[:])

            if rep_cm is not None:
                rep_cm.__exit__(None, None, None)

    nc.compile()
    return nc


def _shard_inputs(state, k1, b1, k2, b2, k3, b3, W, bd):
    """Host-side: build per-core input maps (layout only, no math)."""
    state = np.asarray(state, dtype=np.float32)
    k1 = np.asarray(k1, np.float32); b1 = np.asarray(b1, np.float32)
    k2 = np.asarray(k2, np.float32); b2 = np.asarray(b2, np.float32)
    k3 = np.asarray(k3, np.float32); b3 = np.asarray(b3, np.float32)
    W = np.asarray(W, np.float32)
    W3 = W.reshape(L3, C3 * N_CORES, 2)

    in_maps = []
    for j in range(N_CORES):
        x0 = state[:, :, C0 * j:C0 * (j + 1)]  # [B, L, 2]

        # conv1 im2col a1 [80=(bg,k,c0), (l1, b8)] bf16
        a1 = np.zeros((80, NB1), np.float32)
        for bg in range(BG):
            for k in range(K):
                for c in range(C0):
                    a1[bg * 10 + k * C0 + c] = (
                        x0[bg * B8:(bg + 1) * B8, k:k + L1, c].T.reshape(-1))

        # conv1 stationary blockdiag [80, 128=(c1*8+bg)]
        s1 = np.zeros((80, 128), np.float32)
        for bg in range(BG):
            for c in range(C0):
                for k in range(K):
                    for f in range(FILTERS):
                        c1i = c * FILTERS + f
                        s1[bg * 10 + k * C0 + c,
                           c1i * 8 + bg] = k1[k, 0, (C0 * j + c) * FILTERS + f]
        b1p = np.zeros(128, np.float32)
        for c1i in range(C1):
            b1p[c1i * 8:(c1i + 1) * 8] = b1[C1 * j + c1i]

        # conv2 stationary [80=(c1*5+k), 128 = P(c2)]: out-channel
        # c2 = 8*cc+gg sits at partition P = 4*(cc%8)+(gg%4)+32*(gg//4)
        # +64*(cc//8) so each conv3 group's reads span all 16 ports.
        P2 = np.zeros(128, np.int64)
        for c2i in range(128):
            gg, cc = c2i % 8, c2i // 8
            P2[c2i] = 4 * (cc % 8) + (gg % 4) + 32 * (gg // 4) + 64 * (cc // 8)
        s2 = np.zeros((80, 128), np.float32)
        for k in range(K):
            for c in range(C1):
                for f in range(FILTERS):
                    s2[c * K + k, P2[c * FILTERS + f]] = (
                        k2[k, 0, (C1 * j + c) * FILTERS + f])
        b2p = np.zeros(128, np.float32)
        b2p[P2] = b2[C2 * j:C2 * (j + 1)]

        # conv3 stationaries [80=(c*5+k), 8x128]; group g = {c2: c2%8==g},
        # within-group out col m = c*8+f for c2 = 8c+g
        s3 = np.zeros((80, G3 * 128), np.float32)
        b3p = np.zeros((128, G3), np.float32)
        for g in range(G3):
            for c in range(C1):
                c2l = 8 * c + g
                for k in range(K):
                    for f in range(FILTERS):
                        s3[c * K + k, g * 128 + c * FILTERS + f] = (
                            k3[k, 0, (C2 * j + c2l) * FILTERS + f])
                b3p[c * FILTERS:(c + 1) * FILTERS, g] = b3[
                    (C2 * j + c2l) * FILTERS:(C2 * j + c2l) * FILTERS + FILTERS]

        cs = np.zeros((128, 10), np.float32)
        cs[:, 0] = b1p
        cs[:, 1] = b2p
        cs[:, 2:10] = b3p

        sbm = np.zeros((80, 256 + G3 * 128), np.float32)
        sbm[:, 0:128] = s1
        sbm[:, 128:256] = s2
        sbm[:, 256:] = s3

        # dense weights [128p = m(c,f), (g, lq, q, a)] bf16
        Wj = W3[:, C3 * j:C3 * (j + 1), :]          # [L3, 1024, 2]
        wtm = np.zeros((128, G3, L3 // 4, 4, 2), np.float32)
        for g in range(G3):
            for c in range(C1):
                c2l = 8 * c + g
                for f in range(FILTERS):
                    m = c * FILTERS + f
                    wtm[m, g] = Wj[:, c2l * FILTERS + f, :].reshape(L3 // 4, 4, 2)
        wt = wtm.reshape(128, G3 * L3 * 2).astype(ml_dtypes.bfloat16)

        in_maps.append({"a1": a1.astype(ml_dtypes.bfloat16),
                        "cs": cs, "wt": wt,
                        "sb": sbm.astype(ml_dtypes.bfloat16)})
    return in_maps


def kernel(state, k1, b1, k2, b2, k3, b3, W, bd, **run_kwargs):
    if "nc" not in _CACHE:
        _CACHE["nc"] = _build_nc()
    nc = _CACHE["nc"]
    in_maps = _shard_inputs(state, k1, b1, k2, b2, k3, b3, W, bd)
    res = run_bass_kernel_spmd(nc, in_maps, list(range(N_CORES)), **run_kwargs)
    # device out [128, 256]: cols are (bg 8, q 4, b8 8);
    # partial[a, bg*8+b8] = sum_{tj,q} out[32*tj + 2*q + a, bg*32 + q*8 + b8]
    total = np.zeros((2, B), np.float32)
    for c in range(N_CORES):
        o = np.asarray(res.results[c]["out"]).reshape(128, BG, 4, B8)
        for tj in range(4):
            for q in range(4):
                total += o[32 * tj + 2 * q:32 * tj + 2 * q + 2, :, q, :].reshape(2, B)
    out = np.tanh(total.T + np.asarray(bd, np.float32)).astype(np.float32)
    if run_kwargs.get("trace"):
        _CACHE["last_result"] = res
    return out


# revision 11
# speedup vs baseline: 1.1415x; 1.1415x over previous
"""Trainium2 Bass kernel for nn_Actor (3 grouped conv1d blocks + dense + tanh).

Sharding: column-parallel across 8 cores. Core j owns input channels
{2j, 2j+1}; every conv is grouped (depthwise x8 filters), so that
slice owns contiguous channel blocks through the whole net:
  conv1 out-ch [16j,16j+16), conv2 out-ch [128j,128j+128),
  conv3 out-ch [1024j, 1024j+1024), and rows {l*8192 + ch} of W.
Each core computes partial dense outputs; the host sums them, adds bd
and applies tanh.

Pipeline design (all bf16 compute, f32 psum):
- conv1 out tmp1 [128p = (c1*8 + bg), (l1, b8)].
- i2 [80p = (c1*5 + k), (bg, l2, b8)] via 8 DMAs (one per bg).
- x2r [128p = P2(c2), (bg, l2, b8)] 960-stride cols.
- conv3 im2col i3 [80p = (c*5+k), 7680]: i3[(c,k), col] = x2r[P(c), col+8k]
  (UNtrimmed copy: tap shift is a flat column offset, so each (partition,
  tap) is ONE ~15KB contiguous DMA segment; 10 DMA instructions per group).
  Valid window per bg block: cols bg*960 + [0, 928).
- conv3: per (group, bg): 2 MMs (512+416) into a [128, 1024] 2-bank psum
  tile (3-buf rotation), ONE relu [128, 928] per tile (alternating
  Act/DVE) -> zf [128, 7680] (bufs=4).
- dense: quad-MMs reading zf(g-2) (2-group-old => RAW waits ancient),
  interleaved ~4 per bg-tile so PE wait-processing is always covered by
  wait-free MMs; 4 col-tile accumulators in ONE psum bank.
- software-pipelined loop body: next iteration's conv1/i2/conv2 and
  i3(g0,g1) prefetch are emitted between dense tail groups 6 and 7, so
  the PE stream crosses iteration boundaries without a gap.
"""

import ml_dtypes
import numpy as np

import concourse.bass as bass
import concourse.tile as tile
from concourse import bacc
from concourse import mybir
from concourse.bass_utils import run_bass_kernel_spmd

B = 64
L = 128
C = 16
FILTERS = 8
K = 5
N_CORES = 8

L1 = L - K + 1      # 124
L2 = L1 - K + 1     # 120
L3 = L2 - K + 1     # 116

C0 = C // N_CORES           # 2    input cols per core
C1 = C0 * FILTERS           # 16   conv1 out-ch per core
C2 = C1 * FILTERS           # 128  conv2 out-ch per core
C3 = C2 * FILTERS           # 1024 conv3 out-ch per core
G3 = C2 // C1               # 8    conv3 groups of 16 in-ch
BG = 8                      # batch groups
B8 = B // BG                # 8

NB1 = L1 * B8               # 992   conv1 free (l1, b8)
NB2 = L2 * B                # 7680  conv2 cols (bg, l2, b8)
SB3 = L2 * B8               # 960   per-bg col stride
VB3 = L3 * B8               # 928   valid cols per bg block

F32 = mybir.dt.float32
BF16 = mybir.dt.bfloat16

_CACHE = {}


def _build_nc(reps=1, mode='full'):
    """Build the SPMD Bass program (same on all 8 cores).

    reps>1 wraps the pipeline in a device-side loop (used only for
    timing by wall-clock differencing).
    """
    nc = bacc.Bacc("TRN2", target_bir_lowering=False, debug=False)

    # cs (f32): col0 b1p [(c1*8+bg)], col1 b2p [c2], cols 2..9 b3p per group
    # sb (bf16): s1 [80,128] @0, s2 [80,128] @128, s3 8x[80,128] @256
    a1 = nc.declare_dram_parameter("a1", [80, NB1], BF16, isOutput=False)
    cs = nc.declare_dram_parameter("cs", [128, 10], F32, isOutput=False)
    sb = nc.declare_dram_parameter("sb", [80, 256 + G3 * 128], BF16, isOutput=False)
    wt = nc.declare_dram_parameter("wt", [128, G3 * L3 * 2], BF16, isOutput=False)
    out = nc.declare_dram_parameter("out", [128, 4 * B], F32, isOutput=True)

    do_mm = mode not in ('empty', 'conv2stop', 'i3only')
    do_relu3 = mode not in ('norelu',)
    do_dense = mode not in ('nodense',)

    with tile.TileContext(nc) as tc:
        with (
            tc.tile_pool(name="consts", bufs=1) as consts,
            tc.tile_pool(name="work", bufs=1) as work,
            tc.tile_pool(name="i3pool", bufs=4) as i3pool,
            tc.tile_pool(name="zpool", bufs=4) as zpool,
            tc.tile_pool(name="psum", bufs=3, space=bass.MemorySpace.PSUM) as psum,
            tc.tile_pool(name="psumd", bufs=1, space=bass.MemorySpace.PSUM) as psumd,
        ):
            # ---- constants ----
            a1_t = consts.tile([80, NB1], BF16)
            cs_t = consts.tile([128, 10], F32)
            sb_t = consts.tile([80, 256 + G3 * 128], BF16)
            nc.sync.dma_start(a1_t[:], a1[:])
            nc.scalar.dma_start(cs_t[:], cs[:])
            nc.sync.dma_start(sb_t[:], sb[:])
            WQ = G3 * L3 * 2 // 4  # 464
            wt_ts = []
            for q in range(4):
                wq = consts.tile([128, WQ], BF16, tag=f"wt{q}")
                (nc.scalar if q % 2 else nc.sync).dma_start(
                    wq[:], wt[:, q * WQ:(q + 1) * WQ])
                wt_ts.append(wq)

            # persistent work tiles (single-buffered; WAR handled by sems)
            # x2r padded +32 cols so the tap-shifted i3 reads stay in bounds
            tmp1 = work.tile([128, NB1], BF16)
            i2 = work.tile([80, NB2], BF16)
            x2r_p = work.tile([C2, NB2 + 4 * FILTERS], BF16, name="x2r_p")
            x2r = x2r_p[:, 0:NB2]
            nc.gpsimd.memset(x2r_p[:, NB2:NB2 + 4 * FILTERS], 0.0)

            # DMA ring alternation
            rings = [nc.sync, nc.scalar]
            ring_i = [0]

            def dma(dst, src):
                rings[ring_i[0]].dma_start(dst, src)
                ring_i[0] ^= 1

            # relu engine alternation: Act / DVE
            relu_i = [0]

            def relu(dst, src, bias_ap):
                if relu_i[0] == 0:
                    nc.scalar.activation(dst, src,
                                         mybir.ActivationFunctionType.Relu,
                                         bias=bias_ap)
                else:
                    nc.vector.tensor_scalar(dst, src, bias_ap, 0.0,
                                            mybir.AluOpType.add,
                                            mybir.AluOpType.max)
                relu_i[0] ^= 1

            def prologue():
                """conv1 + i2 im2col + conv2 + issue i3(g0, g1)."""
                # conv1: 2 MMs into one 2-bank psum tile, one relu
                if mode != 'empty':
                    p1 = psum.tile([128, 1024], F32, tag="pchunk", name="p1")
                    nc.tensor.matmul(p1[:, 0:512], sb_t[0:80, 0:128],
                                     a1_t[:, 0:512], start=True, stop=True)
                    nc.tensor.matmul(p1[:, 512:NB1], sb_t[0:80, 0:128],
                                     a1_t[:, 512:NB1], start=True, stop=True)
                    relu(tmp1[:], p1[:, 0:NB1], cs_t[:, 0:1])

                    # i2 im2col: ONE DMA per bg
                    for bg in range(BG):
                        t1b = tmp1[bg:128, 0:NB1]
                        src = bass.AP(t1b.tensor, t1b.offset,
                                      [[t1b.ap[0][0] * 8, C1], [B8, K],
                                       [1, SB3]])
                        dma(i2[0:80, bg * SB3:(bg + 1) * SB3], src)

                    # conv2: 15 MMs, relu per 1024 (512 for the tail)
                    for t in range(8):
                        n = 1024 if t < 7 else 512
                        p2 = psum.tile([128, 1024], F32, tag="pchunk",
                                       name="p2")
                        nc.tensor.matmul(p2[:, 0:512], sb_t[0:80, 128:256],
                                         i2[:, t * 1024:t * 1024 + 512],
                                         start=True, stop=True)
                        if n == 1024:
                            nc.tensor.matmul(
                                p2[:, 512:1024], sb_t[0:80, 128:256],
                                i2[:, t * 1024 + 512:(t + 1) * 1024],
                                start=True, stop=True)
                        relu(x2r[:, t * 1024:t * 1024 + n], p2[:, 0:n],
                             cs_t[:, 1:2])
                if mode not in ('empty', 'conv2stop'):
                    issue_i3(0)
                    issue_i3(1)

            i3s = {}

            def issue_i3(g):
                # 10 DMAs (tap k x half e): dst i3 rows (c*5+k), c=8e+d;
                # i3[(c,k), col] = x2r[P(c), col + 8k], 32 descriptors of
                # 3840B each (x2r is padded so the shifted reads stay in
                # bounds; junk lands only in invalid i3 windows).
                i3 = i3pool.tile([80, NB2], BF16, tag="i3", name="i3")
                xb = x2r_p[0:128, 0:NB2 + 4 * FILTERS]
                xp = xb.ap[0][0]
                ib = i3[0:80, 0:NB2]
                pp = ib.ap[0][0]
                pbase = (g % 4) + 32 * (g // 4)
                for k in range(K):
                    for e in range(2):
                        src = bass.AP(xb.tensor,
                                      xb.offset + (pbase + 64 * e) * xp + 8 * k,
                                      [[4 * xp, 8], [2 * SB3, 4], [1, 2 * SB3]])
                        dst = bass.AP(ib.tensor,
                                      ib.offset + (40 * e + k) * pp,
                                      [[5 * pp, 8], [2 * SB3, 4], [1, 2 * SB3]])
                        dma(dst, src)
                i3s[g] = i3

            # dense: quad-MMs (4 l x 2 actions = 8 psum rows, moving
            # cols (bg, 4, b8) = 256); 4 col-tile accumulators in ONE
            # psum bank (rows 32*tj..32*tj+8, cols 0..256).
            NQ = L3 // 4                 # 29 quads per group
            per_tile = G3 * NQ // 4      # 58 MMs per col-tile
            pd = psumd.tile([128, 512], F32, tag="pd", name="pd")
            tile_seen = [0, 0, 0, 0]
            qcount = [0]

            def emit_dense_quad(zf, g):
                lq = qcount[0] % NQ
                qcount[0] += 1
                zb = zf[0:128, 0:1]
                zp, zf0 = zb.ap[0], zb.offset
                tj = (g * NQ + lq) % 4
                wcol = 232 * (g % 2) + 8 * lq
                mv = bass.AP(zb.tensor, zf0 + 4 * lq * B8,
                             [zp, [SB3, BG], [B8, 4], [1, B8]])
                nc.tensor.matmul(pd[32 * tj:32 * tj + 8, 0:4 * B],
                                 wt_ts[g // 2][:, wcol:wcol + 8],
                                 mv,
                                 start=(tile_seen[tj] == 0),
                                 stop=(tile_seen[tj] == per_tile - 1),
                                 tile_position=(0, 32 * tj))
                tile_seen[tj] += 1

            def conv3_group(g, pend):
                """conv3 MMs + relu for group g; interleave dense quads of
                pend = (zf, g-2)."""
                i3 = i3s.pop(g)
                if do_relu3:
                    zf = zpool.tile([C2, NB2], BF16, tag="zf", name="zf")
                else:
                    zf = x2r  # junk moving data for the norelu timing probe
                # quads per bg slot: 4,4,4,4,4,3,3,3 = 29
                for bg in range(BG):
                    p3 = psum.tile([128, 1024], F32, tag="pchunk", name="p3")
                    c0 = bg * SB3
                    nc.tensor.matmul(p3[:, 0:512],
                                     sb_t[:, 256 + g * 128:256 + (g + 1) * 128],
                                     i3[:, c0:c0 + 512], start=True, stop=True)
                    nc.tensor.matmul(p3[:, 512:VB3],
                                     sb_t[:, 256 + g * 128:256 + (g + 1) * 128],
                                     i3[:, c0 + 512:c0 + VB3],
                                     start=True, stop=True)
                    if do_relu3:
                        relu(zf[:, c0:c0 + VB3], p3[:, 0:VB3],
                             cs_t[:, 2 + g:3 + g])
                    if pend is not None and do_dense:
                        nq = 4 if bg < 5 else 3
                        for _ in range(nq):
                            emit_dense_quad(*pend)
                return zf

            rep_cm = (tc.For_i(0, reps, 1,
                               hint_engines=(mybir.EngineType.PE,
                                             mybir.EngineType.DVE,
                                             mybir.EngineType.Activation,
                                             mybir.EngineType.SP,
                                             mybir.EngineType.Pool))
                      if reps > 1 else None)

            def dense_tail(zf, g):
                for _ in range(NQ):
                    emit_dense_quad(zf, g)

            prologue()
            if rep_cm is not None:
                rep_cm.__enter__()

            if do_mm:
                zfs = {}
                for g in range(G3):
                    if g + 2 < G3 and mode != 'i3only':
                        issue_i3(g + 2)
                    pend = (zfs.pop(g - 2), g - 2) if g >= 2 else None
                    zfs[g] = conv3_group(g, pend)
                if do_dense:
                    dense_tail(zfs[6], 6)
                if rep_cm is not None:
                    prologue()   # next iteration's, overlaps dense tail
                if do_dense:
                    dense_tail(zfs[7], 7)
                zfs.clear()
            else:
                if mode == 'i3only':
                    for g in range(G3 - 2):
                        issue_i3(g + 2)
                    i3s.clear()
                if rep_cm is not None:
                    prologue()

            # ---- write partials ----
            out_t = work.tile([128, 4 * B], F32)
            if mode == 'full':
                for tj in range(4):
                    nc.vector.tensor_copy(out_t[32 * tj:32 * tj + 8, :],
                                          pd[32 * tj:32 * tj + 8, 0:4 * B])
            else:
                nc.gpsimd.memset(out_t[:], 0.0)
            nc.sync.dma_start(out[:], out_t[:])

            if rep_cm is not None:
                rep_cm.__exit__(None, None, None)

    nc.compile()
    return nc


def _shard_inputs(state, k1, b1, k2, b2, k3, b3, W, bd):
    """Host-side: build per-core input maps (layout only, no math)."""
    state = np.asarray(state, dtype=np.float32)
    k1 = np.asarray(k1, np.float32); b1 = np.asarray(b1, np.float32)
    k2 = np.asarray(k2, np.float32); b2 = np.asarray(b2, np.float32)
    k3 = np.asarray(k3, np.float32); b3 = np.asarray(b3, np.float32)
    W = np.asarray(W, np.float32)
    W3 = W.reshape(L3, C3 * N_CORES, 2)

    in_maps = []
    for j in range(N_CORES):
        x0 = state[:, :, C0 * j:C0 * (j + 1)]  # [B, L, 2]

        # conv1 im2col a1 [80=(bg,k,c0), (l1, b8)] bf16
        a1 = np.zeros((80, NB1), np.float32)
        for bg in range(BG):
            for k in range(K):
                for c in range(C0):
                    a1[bg * 10 + k * C0 + c] = (
                        x0[bg * B8:(bg + 1) * B8, k:k + L1, c].T.reshape(-1))

        # conv1 stationary blockdiag [80, 128=(c1*8+bg)]
        s1 = np.zeros((80, 128), np.float32)
        for bg in range(BG):
            for c in range(C0):
                for k in range(K):
                    for f in range(FILTERS):
                        c1i = c * FILTERS + f
                        s1[bg * 10 + k * C0 + c,
                           c1i * 8 + bg] = k1[k, 0, (C0 * j + c) * FILTERS + f]
        b1p = np.zeros(128, np.float32)
        for c1i in range(C1):
            b1p[c1i * 8:(c1i + 1) * 8] = b1[C1 * j + c1i]

        # conv2 stationary [80=(c1*5+k), 128 = P(c2)]: out-channel
        # c2 = 8*cc+gg sits at partition P = 4*(cc%8)+(gg%4)+32*(gg//4)
        # +64*(cc//8) so each conv3 group's reads span all 16 ports.
        P2 = np.zeros(128, np.int64)
        for c2i in range(128):
            gg, cc = c2i % 8, c2i // 8
            P2[c2i] = 4 * (cc % 8) + (gg % 4) + 32 * (gg // 4) + 64 * (cc // 8)
        s2 = np.zeros((80, 128), np.float32)
        for k in range(K):
            for c in range(C1):
                for f in range(FILTERS):
                    s2[c * K + k, P2[c * FILTERS + f]] = (
                        k2[k, 0, (C1 * j + c) * FILTERS + f])
        b2p = np.zeros(128, np.float32)
        b2p[P2] = b2[C2 * j:C2 * (j + 1)]

        # conv3 stationaries [80=(c*5+k), 8x128]; group g = {c2: c2%8==g},
        # within-group out col m = c*8+f for c2 = 8c+g
        s3 = np.zeros((80, G3 * 128), np.float32)
        b3p = np.zeros((128, G3), np.float32)
        for g in range(G3):
            for c in range(C1):
                c2l = 8 * c + g
                for k in range(K):
                    for f in range(FILTERS):
                        s3[c * K + k, g * 128 + c * FILTERS + f] = (
                            k3[k, 0, (C2 * j + c2l) * FILTERS + f])
                b3p[c * FILTERS:(c + 1) * FILTERS, g] = b3[
                    (C2 * j + c2l) * FILTERS:(C2 * j + c2l) * FILTERS + FILTERS]

        cs = np.zeros((128, 10), np.float32)
        cs[:, 0] = b1p
        cs[:, 1] = b2p
        cs[:, 2:10] = b3p

        sbm = np.zeros((80, 256 + G3 * 128), np.float32)
        sbm[:, 0:128] = s1
        sbm[:, 128:256] = s2
        sbm[:, 256:] = s3

        # dense weights [128p = m(c,f), (g, lq, q, a)] bf16
        Wj = W3[:, C3 * j:C3 * (j + 1), :]          # [L3, 1024, 2]
        wtm = np.zeros((128, G3, L3 // 4, 4, 2), np.float32)
        for g in range(G3):
            for c in range(C1):
                c2l = 8 * c + g
                for f in range(FILTERS):
                    m = c * FILTERS + f
                    wtm[m, g] = Wj[:, c2l * FILTERS + f, :].reshape(L3 // 4, 4, 2)
        wt = wtm.reshape(128, G3 * L3 * 2).astype(ml_dtypes.bfloat16)

        in_maps.append({"a1": a1.astype(ml_dtypes.bfloat16),
                        "cs": cs, "wt": wt,
                        "sb": sbm.astype(ml_dtypes.bfloat16)})
    return in_maps


def kernel(state, k1, b1, k2, b2, k3, b3, W, bd, **run_kwargs):
    if "nc" not in _CACHE:
        _CACHE["nc"] = _build_nc()
    nc = _CACHE["nc"]
    in_maps = _shard_inputs(state, k1, b1, k2, b2, k3, b3, W, bd)
    res = run_bass_kernel_spmd(nc, in_maps, list(range(N_CORES)), **run_kwargs)
    # device out [128, 256]: cols are (bg 8, q 4, b8 8);
    # partial[a, bg*8+b8] = sum_{tj,q} out[32*tj + 2*q + a, bg*32 + q*8 + b8]
    total = np.zeros((2, B), np.float32)
    for c in range(N_CORES):
        o = np.asarray(res.results[c]["out"]).reshape(128, BG, 4, B8)
        for tj in range(4):
            for q in range(4):
                total += o[32 * tj + 2 * q:32 * tj + 2 * q + 2, :, q, :].reshape(2, B)
    out = np.tanh(total.T + np.asarray(bd, np.float32)).astype(np.float32)
    if run_kwargs.get("trace"):
        _CACHE["last_result"] = res
    return out


# revision 12
# speedup vs baseline: 1.1897x; 1.0422x over previous
"""Trainium2 Bass kernel for nn_Actor (3 grouped conv1d blocks + dense + tanh).

Sharding: column-parallel across 8 cores. Core j owns input channels
{2j, 2j+1}; every conv is grouped (depthwise x8 filters), so that
slice owns contiguous channel blocks through the whole net:
  conv1 out-ch [16j,16j+16), conv2 out-ch [128j,128j+128),
  conv3 out-ch [1024j, 1024j+1024), and rows {l*8192 + ch} of W.
Each core computes partial dense outputs; the host sums them, adds bd
and applies tanh.

Pipeline design (all bf16 compute, f32 psum):
- conv1 out tmp1 [128p = (c1*8 + bg), (l1, b8)].
- i2 [80p = (c1*5 + k), (bg, l2, b8)] via 8 DMAs (one per bg).
- x2r [128p = P2(c2), (bg, l2, b8)] 960-stride cols.
- conv3 im2col i3 [80p = (c*5+k), 7680]: i3[(c,k), col] = x2r[P(c), col+8k]
  (UNtrimmed copy: tap shift is a flat column offset, so each (partition,
  tap) is ONE ~15KB contiguous DMA segment; 10 DMA instructions per group).
  Valid window per bg block: cols bg*960 + [0, 928).
- conv3: per (group, bg): 2 MMs (512+416) into a [128, 1024] 2-bank psum
  tile (3-buf rotation), ONE relu [128, 928] per tile (alternating
  Act/DVE) -> zf [128, 7680] (bufs=4).
- dense: quad-MMs reading zf(g-2) (2-group-old => RAW waits ancient),
  interleaved ~4 per bg-tile so PE wait-processing is always covered by
  wait-free MMs; 4 col-tile accumulators in ONE psum bank.
- software-pipelined loop body: next iteration's conv1/i2/conv2 and
  i3(g0,g1) prefetch are emitted between dense tail groups 6 and 7, so
  the PE stream crosses iteration boundaries without a gap.
"""

import ml_dtypes
import numpy as np

import concourse.bass as bass
import concourse.tile as tile
from concourse import bacc
from concourse import mybir
from concourse.bass_utils import run_bass_kernel_spmd

B = 64
L = 128
C = 16
FILTERS = 8
K = 5
N_CORES = 8

L1 = L - K + 1      # 124
L2 = L1 - K + 1     # 120
L3 = L2 - K + 1     # 116

C0 = C // N_CORES           # 2    input cols per core
C1 = C0 * FILTERS           # 16   conv1 out-ch per core
C2 = C1 * FILTERS           # 128  conv2 out-ch per core
C3 = C2 * FILTERS           # 1024 conv3 out-ch per core
G3 = C2 // C1               # 8    conv3 groups of 16 in-ch
BG = 8                      # batch groups
B8 = B // BG                # 8

NB1 = L1 * B8               # 992   conv1 free (l1, b8)
NB2 = L2 * B                # 7680  conv2 cols (bg, l2, b8)
SB3 = L2 * B8               # 960   per-bg col stride
VB3 = L3 * B8               # 928   valid cols per bg block

F32 = mybir.dt.float32
BF16 = mybir.dt.bfloat16

_CACHE = {}


def _build_nc(reps=1, mode='full'):
    """Build the SPMD Bass program (same on all 8 cores).

    reps>1 wraps the pipeline in a device-side loop (used only for
    timing by wall-clock differencing).
    """
    nc = bacc.Bacc("TRN2", target_bir_lowering=False, debug=False)

    # cs (f32): col0 b1p [(c1*8+bg)], col1 b2p [c2], cols 2..9 b3p per group
    # sb (bf16): s1 [80,128] @0, s2 [80,128] @128, s3 8x[80,128] @256
    a1 = nc.declare_dram_parameter("a1", [80, NB1], BF16, isOutput=False)
    cs = nc.declare_dram_parameter("cs", [128, 10], F32, isOutput=False)
    sb = nc.declare_dram_parameter("sb", [80, 256 + G3 * 128], BF16, isOutput=False)
    wt = nc.declare_dram_parameter("wt", [128, G3 * L3 * 2], BF16, isOutput=False)
    out = nc.declare_dram_parameter("out", [128, 4 * B], F32, isOutput=True)

    do_mm = mode not in ('empty', 'conv2stop', 'i3only')
    do_relu3 = mode not in ('norelu',)
    do_dense = mode not in ('nodense',)

    with tile.TileContext(nc) as tc:
        with (
            tc.tile_pool(name="consts", bufs=1) as consts,
            tc.tile_pool(name="work", bufs=1) as work,
            tc.tile_pool(name="i3pool", bufs=4) as i3pool,
            tc.tile_pool(name="zpool", bufs=4) as zpool,
            tc.tile_pool(name="psum", bufs=3, space=bass.MemorySpace.PSUM) as psum,
            tc.tile_pool(name="psumd", bufs=1, space=bass.MemorySpace.PSUM) as psumd,
        ):
            # ---- constants ----
            a1_t = consts.tile([80, NB1], BF16)
            cs_t = consts.tile([128, 10], F32)
            sb_t = consts.tile([80, 256 + G3 * 128], BF16)
            nc.sync.dma_start(a1_t[:], a1[:])
            nc.scalar.dma_start(cs_t[:], cs[:])
            nc.sync.dma_start(sb_t[:], sb[:])
            WQ = G3 * L3 * 2 // 4  # 464
            wt_ts = []
            for q in range(4):
                wq = consts.tile([128, WQ], BF16, tag=f"wt{q}")
                (nc.scalar if q % 2 else nc.sync).dma_start(
                    wq[:], wt[:, q * WQ:(q + 1) * WQ])
                wt_ts.append(wq)

            # persistent work tiles (single-buffered; WAR handled by sems)
            # x2r padded +32 cols so the tap-shifted i3 reads stay in bounds
            tmp1 = work.tile([128, NB1], BF16)
            i2 = work.tile([80, NB2], BF16)
            x2r_p = work.tile([C2, NB2 + 4 * FILTERS], BF16, name="x2r_p")
            x2r = x2r_p[:, 0:NB2]
            nc.gpsimd.memset(x2r_p[:, NB2:NB2 + 4 * FILTERS], 0.0)

            # DMA ring alternation
            rings = [nc.sync, nc.scalar]
            ring_i = [0]

            def dma(dst, src):
                rings[ring_i[0]].dma_start(dst, src)
                ring_i[0] ^= 1

            # relu engine alternation: Act / DVE
            relu_i = [0]

            def relu(dst, src, bias_ap):
                if relu_i[0] == 0:
                    nc.scalar.activation(dst, src,
                                         mybir.ActivationFunctionType.Relu,
                                         bias=bias_ap)
                else:
                    nc.vector.tensor_scalar(dst, src, bias_ap, 0.0,
                                            mybir.AluOpType.add,
                                            mybir.AluOpType.max)
                relu_i[0] ^= 1

            def prologue():
                """conv1 + i2 im2col + conv2 + issue i3(g0, g1)."""
                # conv1: 2 MMs into one 2-bank psum tile, one relu
                if mode != 'empty':
                    p1 = psum.tile([128, 1024], F32, tag="pchunk", name="p1")
                    nc.tensor.matmul(p1[:, 0:512], sb_t[0:80, 0:128],
                                     a1_t[:, 0:512], start=True, stop=True)
                    nc.tensor.matmul(p1[:, 512:NB1], sb_t[0:80, 0:128],
                                     a1_t[:, 512:NB1], start=True, stop=True)
                    relu(tmp1[:], p1[:, 0:NB1], cs_t[:, 0:1])

                    # i2 im2col: ONE DMA per bg
                    for bg in range(BG):
                        t1b = tmp1[bg:128, 0:NB1]
                        src = bass.AP(t1b.tensor, t1b.offset,
                                      [[t1b.ap[0][0] * 8, C1], [B8, K],
                                       [1, SB3]])
                        dma(i2[0:80, bg * SB3:(bg + 1) * SB3], src)

                    # conv2: 15 MMs, relu per 1024 (512 for the tail)
                    for t in range(8):
                        n = 1024 if t < 7 else 512
                        p2 = psum.tile([128, 1024], F32, tag="pchunk",
                                       name="p2")
                        nc.tensor.matmul(p2[:, 0:512], sb_t[0:80, 128:256],
                                         i2[:, t * 1024:t * 1024 + 512],
                                         start=True, stop=True)
                        if n == 1024:
                            nc.tensor.matmul(
                                p2[:, 512:1024], sb_t[0:80, 128:256],
                                i2[:, t * 1024 + 512:(t + 1) * 1024],
                                start=True, stop=True)
                        relu(x2r[:, t * 1024:t * 1024 + n], p2[:, 0:n],
                             cs_t[:, 1:2])
                if mode not in ('empty', 'conv2stop'):
                    issue_i3(0)
                    issue_i3(1)

            i3s = {}

            def issue_i3(g):
                # 10 DMAs (tap k x half e): dst i3 rows (c*5+k), c=8e+d;
                # i3[(c,k), col] = x2r[P(c), col + 8k], 32 descriptors of
                # 3840B each (x2r is padded so the shifted reads stay in
                # bounds; junk lands only in invalid i3 windows).
                i3 = i3pool.tile([80, NB2], BF16, tag="i3", name="i3")
                xb = x2r_p[0:128, 0:NB2 + 4 * FILTERS]
                xp = xb.ap[0][0]
                ib = i3[0:80, 0:NB2]
                pp = ib.ap[0][0]
                pbase = (g % 4) + 32 * (g // 4)
                for k in range(K):
                    for e in range(2):
                        src = bass.AP(xb.tensor,
                                      xb.offset + (pbase + 64 * e) * xp + 8 * k,
                                      [[4 * xp, 8], [2 * SB3, 4], [1, 2 * SB3]])
                        dst = bass.AP(ib.tensor,
                                      ib.offset + (40 * e + k) * pp,
                                      [[5 * pp, 8], [2 * SB3, 4], [1, 2 * SB3]])
                        dma(dst, src)
                i3s[g] = i3

            # dense: quad-MMs (4 l x 2 actions = 8 psum rows, moving
            # cols (bg, 4, b8) = 256); 4 col-tile accumulators in ONE
            # psum bank (rows 32*tj..32*tj+8, cols 0..256).
            NQ = L3 // 4                 # 29 quads per group
            per_tile = G3 * NQ // 4      # 58 MMs per col-tile
            pd = psumd.tile([128, 512], F32, tag="pd", name="pd")
            tile_seen = [0, 0, 0, 0]
            qcount = [0]

            def emit_dense_quad(zf, g):
                lq = qcount[0] % NQ
                qcount[0] += 1
                zb = zf[0:128, 0:1]
                zp, zf0 = zb.ap[0], zb.offset
                tj = (g * NQ + lq) % 4
                wcol = 232 * (g % 2) + 8 * lq
                mv = bass.AP(zb.tensor, zf0 + 4 * lq * B8,
                             [zp, [SB3, BG], [B8, 4], [1, B8]])
                nc.tensor.matmul(pd[32 * tj:32 * tj + 8, 0:4 * B],
                                 wt_ts[g // 2][:, wcol:wcol + 8],
                                 mv,
                                 start=(tile_seen[tj] == 0),
                                 stop=(tile_seen[tj] == per_tile - 1),
                                 tile_position=(0, 32 * tj))
                tile_seen[tj] += 1

            def conv3_group(g, pend):
                """conv3 MMs + relu for group g; interleave dense quads of
                pend = (zf, g-2)."""
                i3 = i3s.pop(g)
                if do_relu3:
                    zf = zpool.tile([C2, NB2], BF16, tag="zf", name="zf")
                else:
                    zf = x2r  # junk moving data for the norelu timing probe
                # quads per bg slot: 4,4,4,4,4,3,3,3 = 29
                for bg in range(BG):
                    p3 = psum.tile([128, 1024], F32, tag="pchunk", name="p3")
                    c0 = bg * SB3
                    nc.tensor.matmul(p3[:, 0:512],
                                     sb_t[:, 256 + g * 128:256 + (g + 1) * 128],
                                     i3[:, c0:c0 + 512], start=True, stop=True)
                    nc.tensor.matmul(p3[:, 512:VB3],
                                     sb_t[:, 256 + g * 128:256 + (g + 1) * 128],
                                     i3[:, c0 + 512:c0 + VB3],
                                     start=True, stop=True)
                    if do_relu3:
                        relu(zf[:, c0:c0 + VB3], p3[:, 0:VB3],
                             cs_t[:, 2 + g:3 + g])
                    if pend is not None and do_dense:
                        nq = 4 if bg < 5 else 3
                        for _ in range(nq):
                            emit_dense_quad(*pend)
                return zf

            rep_cm = (tc.For_i(0, reps, 1,
                               hint_engines=(mybir.EngineType.PE,
                                             mybir.EngineType.DVE,
                                             mybir.EngineType.Activation,
                                             mybir.EngineType.SP,
                                             mybir.EngineType.Pool))
                      if reps > 1 else None)

            def dense_tail(zf, g):
                for _ in range(NQ):
                    emit_dense_quad(zf, g)

            prologue()
            if rep_cm is not None:
                rep_cm.__enter__()

            if do_mm:
                zfs = {}
                for g in range(G3):
                    if g + 2 < G3 and mode != 'i3only':
                        issue_i3(g + 2)
                    if g == 6 and rep_cm is not None:
                        # next iteration's prologue: keeps the DMA queues
                        # fed across the loop turn (i3 buf rotation stays
                        # aligned: issues are g2..g7 then g0', g1')
                        prologue()
                    pend = (zfs.pop(g - 2), g - 2) if g >= 2 else None
                    zfs[g] = conv3_group(g, pend)
                if do_dense:
                    dense_tail(zfs[6], 6)
                    dense_tail(zfs[7], 7)
                zfs.clear()
            else:
                if mode == 'i3only':
                    for g in range(G3 - 2):
                        issue_i3(g + 2)
                    i3s.clear()
                if rep_cm is not None:
                    prologue()

            # ---- write partials ----
            out_t = work.tile([128, 4 * B], F32)
            if mode == 'full':
                for tj in range(4):
                    nc.vector.tensor_copy(out_t[32 * tj:32 * tj + 8, :],
                                          pd[32 * tj:32 * tj + 8, 0:4 * B])
            else:
                nc.gpsimd.memset(out_t[:], 0.0)
            nc.sync.dma_start(out[:], out_t[:])

            if rep_cm is not None:
                rep_cm.__exit__(None, None, None)

    nc.compile()
    return nc


def _shard_inputs(state, k1, b1, k2, b2, k3, b3, W, bd):
    """Host-side: build per-core input maps (layout only, no math)."""
    state = np.asarray(state, dtype=np.float32)
    k1 = np.asarray(k1, np.float32); b1 = np.asarray(b1, np.float32)
    k2 = np.asarray(k2, np.float32); b2 = np.asarray(b2, np.float32)
    k3 = np.asarray(k3, np.float32); b3 = np.asarray(b3, np.float32)
    W = np.asarray(W, np.float32)
    W3 = W.reshape(L3, C3 * N_CORES, 2)

    in_maps = []
    for j in range(N_CORES):
        x0 = state[:, :, C0 * j:C0 * (j + 1)]  # [B, L, 2]

        # conv1 im2col a1 [80=(bg,k,c0), (l1, b8)] bf16
        a1 = np.zeros((80, NB1), np.float32)
        for bg in range(BG):
            for k in range(K):
                for c in range(C0):
                    a1[bg * 10 + k * C0 + c] = (
                        x0[bg * B8:(bg + 1) * B8, k:k + L1, c].T.reshape(-1))

        # conv1 stationary blockdiag [80, 128=(c1*8+bg)]
        s1 = np.zeros((80, 128), np.float32)
        for bg in range(BG):
            for c in range(C0):
                for k in range(K):
                    for f in range(FILTERS):
                        c1i = c * FILTERS + f
                        s1[bg * 10 + k * C0 + c,
                           c1i * 8 + bg] = k1[k, 0, (C0 * j + c) * FILTERS + f]
        b1p = np.zeros(128, np.float32)
        for c1i in range(C1):
            b1p[c1i * 8:(c1i + 1) * 8] = b1[C1 * j + c1i]

        # conv2 stationary [80=(c1*5+k), 128 = P(c2)]: out-channel
        # c2 = 8*cc+gg sits at partition P = 4*(cc%8)+(gg%4)+32*(gg//4)
        # +64*(cc//8) so each conv3 group's reads span all 16 ports.
        P2 = np.zeros(128, np.int64)
        for c2i in range(128):
            gg, cc = c2i % 8, c2i // 8
            P2[c2i] = 4 * (cc % 8) + (gg % 4) + 32 * (gg // 4) + 64 * (cc // 8)
        s2 = np.zeros((80, 128), np.float32)
        for k in range(K):
            for c in range(C1):
                for f in range(FILTERS):
                    s2[c * K + k, P2[c * FILTERS + f]] = (
                        k2[k, 0, (C1 * j + c) * FILTERS + f])
        b2p = np.zeros(128, np.float32)
        b2p[P2] = b2[C2 * j:C2 * (j + 1)]

        # conv3 stationaries [80=(c*5+k), 8x128]; group g = {c2: c2%8==g},
        # within-group out col m = c*8+f for c2 = 8c+g
        s3 = np.zeros((80, G3 * 128), np.float32)
        b3p = np.zeros((128, G3), np.float32)
        for g in range(G3):
            for c in range(C1):
                c2l = 8 * c + g
                for k in range(K):
                    for f in range(FILTERS):
                        s3[c * K + k, g * 128 + c * FILTERS + f] = (
                            k3[k, 0, (C2 * j + c2l) * FILTERS + f])
                b3p[c * FILTERS:(c + 1) * FILTERS, g] = b3[
                    (C2 * j + c2l) * FILTERS:(C2 * j + c2l) * FILTERS + FILTERS]

        cs = np.zeros((128, 10), np.float32)
        cs[:, 0] = b1p
        cs[:, 1] = b2p
        cs[:, 2:10] = b3p

        sbm = np.zeros((80, 256 + G3 * 128), np.float32)
        sbm[:, 0:128] = s1
        sbm[:, 128:256] = s2
        sbm[:, 256:] = s3

        # dense weights [128p = m(c,f), (g, lq, q, a)] bf16
        Wj = W3[:, C3 * j:C3 * (j + 1), :]          # [L3, 1024, 2]
        wtm = np.zeros((128, G3, L3 // 4, 4, 2), np.float32)
        for g in range(G3):
            for c in range(C1):
                c2l = 8 * c + g
                for f in range(FILTERS):
                    m = c * FILTERS + f
                    wtm[m, g] = Wj[:, c2l * FILTERS + f, :].reshape(L3 // 4, 4, 2)
        wt = wtm.reshape(128, G3 * L3 * 2).astype(ml_dtypes.bfloat16)

        in_maps.append({"a1": a1.astype(ml_dtypes.bfloat16),
                        "cs": cs, "wt": wt,
                        "sb": sbm.astype(ml_dtypes.bfloat16)})
    return in_maps


def kernel(state, k1, b1, k2, b2, k3, b3, W, bd, **run_kwargs):
    if "nc" not in _CACHE:
        _CACHE["nc"] = _build_nc()
    nc = _CACHE["nc"]
    in_maps = _shard_inputs(state, k1, b1, k2, b2, k3, b3, W, bd)
    res = run_bass_kernel_spmd(nc, in_maps, list(range(N_CORES)), **run_kwargs)
    # device out [128, 256]: cols are (bg 8, q 4, b8 8);
    # partial[a, bg*8+b8] = sum_{tj,q} out[32*tj + 2*q + a, bg*32 + q*8 + b8]
    total = np.zeros((2, B), np.float32)
    for c in range(N_CORES):
        o = np.asarray(res.results[c]["out"]).reshape(128, BG, 4, B8)
        for tj in range(4):
            for q in range(4):
                total += o[32 * tj + 2 * q:32 * tj + 2 * q + 2, :, q, :].reshape(2, B)
    out = np.tanh(total.T + np.asarray(bd, np.float32)).astype(np.float32)
    if run_kwargs.get("trace"):
        _CACHE["last_result"] = res
    return out
